# revision 2
# baseline (speedup 1.0000x reference)
"""DKVMN forward Trainium2 Bass kernel (fp16 bulk path, v2).

Model (per sample): embeddings -> softmax attention w over M memory slots ->
sequential memory update Mv_t = Mv_{t-1} * (1 - w_t e_t^T) + w_t a_t^T ->
weighted read of PRE-update memory -> output MLP -> sigmoid.

Sharding: data-parallel over batch. B=64 across 8 cores -> 8 samples/core.
Tables + weights replicated. Each core returns [8, 199]; host concatenates.

v2 structure (per core; engine-balanced against the TimelineSim cost model):
- natural-form scan (no sign trick): state = (d0 * state) + d1 with
  d0 = 1 - W*e, d1 = W*a; m-blocks chained in one scan instr per group via
  reset columns (d0=0, d1=Mv0 -> state resets to Mv0).
- sigmoid via tanh: sigmoid(x) = 0.5*tanh(x/2)+0.5 so every Act func
  (Tanh/Exp/Copy) lives in one act table -> no LoadActFuncSet swaps.
  The output affine folds into cheap DVE tensor_scalar (4x mode) ops.
- engine split: scans+NW+G on DVE, BN on Pool, 1-We affine + Wbc
  PSUM->SBUF copies on Act, broadcast + all matmuls incl. the 50-term
  m-reduction on PE (Ldweights free, PSUM accumulation).
- software pipelining: emit P1(b+1) (gather/emb/softmax/Wbc) before
  P2(b) (scan chain + readout) so each engine's in-order stream always
  has sample b+1 front-end work before sample b's back-end waits.
"""
import sys

sys.path.insert(0, "/opt/trn_rl_repo")

import numpy as np

import concourse.bacc as bacc
import concourse.bass as bass
import concourse.tile as tile
from concourse import library_config, mybir
from concourse.bass_utils import run_bass_kernel_spmd

f32 = mybir.dt.float32
f16 = mybir.dt.float16
i16 = mybir.dt.int16
AF = mybir.ActivationFunctionType
ALU = mybir.AluOpType
AX = mybir.AxisListType

B, L, NS, D, M = 64, 200, 1000, 128, 50
NCORES = 8
BL = B // NCORES          # samples per core
NIDX = 256                # padded gather idxs per sample (L=200 real)
MGRP = 10                 # m's per scan group
NGRP = M // MGRP          # 5 groups
GCOLS = MGRP * L          # 2000 w-cols per group
SCOLS = MGRP * (L + 1)    # 2010 scan cols (incl. reset col per m)
WCOLS = M * L             # 10000
BCW = 500                 # broadcast chunk cols
NBC = WCOLS // BCW        # 20 chunks

TRACE = False
LAST_RESULTS = None


def _ap(t_ap, offset_add, free_dims):
    """Raw AP view: keep partition dim, replace free dims."""
    return bass.AP(t_ap.tensor, t_ap.offset + offset_add,
                   [t_ap.ap[0]] + free_dims)


def build_bass(n_samples=BL):
    BLn = n_samples
    nc = bacc.Bacc("TRN2", target_bir_lowering=False, debug=False,
                   num_devices=NCORES)

    def dram_in(name, shape, dtype=f32):
        return nc.dram_tensor(name, shape, dtype, kind="ExternalInput")

    k_emb = dram_in("k_emb", [NS, D], f16)
    v_emb = dram_in("v_emb", [2 * NS, D], f16)
    kidx = dram_in("kidx", [128, BLn * NIDX // 16], i16)
    vidx = dram_in("vidx", [128, BLn * NIDX // 16], i16)
    MkT = dram_in("MkT", [D, M], f16)
    eWT = dram_in("eWT", [D, D], f16)
    aWT = dram_in("aWT", [D, D], f16)
    fWrT = dram_in("fWrT", [D, D], f16)
    fWkT = dram_in("fWkT", [D, D], f16)
    pWT = dram_in("pWT", [D, 1], f16)
    Mv0T16 = dram_in("Mv0T16", [D, M], f16)
    ident = dram_in("ident", [D, D], f16)
    ones16 = dram_in("ones16", [1, D], f16)
    eb2 = dram_in("eb2", [D, 1])            # e_b / 2
    a_b = dram_in("a_b", [D, 1])
    f_b = dram_in("f_b", [D, 1])
    pb2 = dram_in("pb2", [1, 1])            # p_b / 2
    p_out = nc.dram_tensor("p_out", [BLn, L - 1], f32, kind="ExternalOutput")

    with tile.TileContext(nc) as tc:
        nc.gpsimd.load_library(library_config.mlp)
        with tc.tile_pool(name="const", bufs=1) as cpool, \
             tc.tile_pool(name="rows", bufs=2) as rpool, \
             tc.tile_pool(name="sm", bufs=2) as sm, \
             tc.tile_pool(name="wfp", bufs=2) as wfp, \
             tc.tile_pool(name="wbcp", bufs=2) as wbcp, \
             tc.tile_pool(name="unit", bufs=3) as unit, \
             tc.tile_pool(name="cc", bufs=2) as ccp, \
             tc.tile_pool(name="psE", bufs=1, space="PSUM") as psE, \
             tc.tile_pool(name="psA", bufs=1, space="PSUM") as psA, \
             tc.tile_pool(name="psF", bufs=2, space="PSUM") as psF, \
             tc.tile_pool(name="psP", bufs=1, space="PSUM") as psP, \
             tc.tile_pool(name="psW", bufs=1, space="PSUM") as psW, \
             tc.tile_pool(name="psBC", bufs=2, space="PSUM") as psBC:

            def cload(dram, shape, dtype=f32):
                t = cpool.tile(shape, dtype, tag=dram.name)
                nc.sync.dma_start(t[:], dram[(slice(None),) * len(shape)])
                return t

            c_MkT = cload(MkT, [D, M], f16)
            c_eWT = cload(eWT, [D, D], f16)
            c_aWT = cload(aWT, [D, D], f16)
            c_fWrT = cload(fWrT, [D, D], f16)
            c_fWkT = cload(fWkT, [D, D], f16)
            c_pWT = cload(pWT, [D, 1], f16)
            c_Mv0 = cload(Mv0T16, [D, M], f16)
            c_id = cload(ident, [D, D], f16)
            c_ones = cload(ones16, [1, D], f16)
            c_eb2 = cload(eb2, [D, 1])
            c_ab = cload(a_b, [D, 1])
            c_fb = cload(f_b, [D, 1])
            c_pb2 = cload(pb2, [1, 1])
            c_kidx = cload(kidx, [128, BLn * NIDX // 16], i16)
            c_vidx = cload(vidx, [128, BLn * NIDX // 16], i16)

            p_row = sm.tile([1, BLn * L], f32, tag="p_row")

            state = {}

            def phase1(b):
                isl = slice(b * (NIDX // 16), (b + 1) * (NIDX // 16))

                # ---- transposed gathers: directly [D, t] fp16 ----
                kT3 = rpool.tile([128, 1, NIDX], f16, tag="kT3")
                nc.gpsimd.dma_gather(kT3[:], k_emb[:, :], c_kidx[:, isl],
                                     num_idxs=NIDX, num_idxs_reg=L,
                                     elem_size=D, transpose=True)
                vT3 = rpool.tile([128, 1, NIDX], f16, tag="vT3")
                nc.gpsimd.dma_gather(vT3[:], v_emb[:, :], c_vidx[:, isl],
                                     num_idxs=NIDX, num_idxs_reg=L,
                                     elem_size=D, transpose=True)
                kT = _ap(kT3[:], 0, [[1, L]])
                vT = _ap(vT3[:], 0, [[1, L]])

                # ---- h = sigmoid(e_W v + e_b) via tanh; a = tanh(...) ----
                eps = psE.tile([D, L], f32, tag="eps")
                nc.tensor.matmul(eps[:], c_eWT[:], vT)
                th_e = sm.tile([D, L], f16, tag="th_e")
                nc.scalar.activation(th_e[:], eps[:], AF.Tanh,
                                     bias=c_eb2[:], scale=0.5)
                h_T = sm.tile([D, L], f16, tag="h_T")
                nc.vector.tensor_scalar(h_T[:], th_e[:], 0.5, 0.5,
                                        ALU.mult, ALU.add)
                aps = psA.tile([D, L], f32, tag="aps")
                nc.tensor.matmul(aps[:], c_aWT[:], vT)
                a_T = sm.tile([D, L], f16, tag="a_T")
                nc.scalar.activation(a_T[:], aps[:], AF.Tanh,
                                     bias=c_ab[:], scale=1.0)

                # ---- w softmax (f32 psum) -> fp16 [m, t] ----
                wmT = sm.tile([M, L], f16, tag="wmT")
                for tb in range(2):
                    t0 = tb * 128
                    tsz = min(128, L - t0)
                    wps = psW.tile([128, M], f32, tag="wps")
                    nc.tensor.matmul(wps[0:tsz, :],
                                     _ap(kT3[:], t0, [[1, tsz]]),
                                     c_MkT[:])
                    negmax = sm.tile([128, 1], f32, tag="negmax")
                    nc.vector.tensor_reduce(negmax[0:tsz, :], wps[0:tsz, :],
                                            AX.X, ALU.max, negate=True)
                    wexp = sm.tile([128, M], f32, tag="wexp")
                    nc.scalar.activation(wexp[0:tsz, :], wps[0:tsz, :],
                                         AF.Exp, bias=negmax[0:tsz, :],
                                         scale=1.0)
                    ssum = sm.tile([128, 1], f32, tag="ssum")
                    nc.vector.tensor_reduce(ssum[0:tsz, :], wexp[0:tsz, :],
                                            AX.X, ALU.add)
                    rcp = sm.tile([128, 1], f32, tag="rcp")
                    nc.vector.reciprocal(rcp[0:tsz, :], ssum[0:tsz, :])
                    w16 = sm.tile([128, M], f16, tag="w16")
                    nc.vector.tensor_scalar_mul(w16[0:tsz, :], wexp[0:tsz, :],
                                                rcp[0:tsz, :])
                    wtp = psW.tile([M, 128], f16, tag="wps")
                    nc.tensor.transpose(wtp[:, 0:tsz], w16[0:tsz, :],
                                        c_id[0:tsz, 0:tsz])
                    nc.vector.tensor_copy(wmT[:, t0:t0 + tsz],
                                          wtp[:, 0:tsz])

                # ---- w_flat [1, M*L] fp16 (m-major) -> Wbc via PE + ACT ----
                w_flat = wfp.tile([1, WCOLS], f16, tag="w_flat")
                nc.sync.dma_start(
                    _ap(w_flat[:], 0, [[L, M], [1, L]]), wmT[:])
                Wbc = wbcp.tile([128, WCOLS], f16, tag="Wbc")
                for cch in range(NBC):
                    bps = psBC.tile([128, BCW + 12], f32, tag="bc")
                    nc.tensor.matmul(
                        bps[:, 0:BCW], c_ones[:],
                        w_flat[0:1, cch * BCW:(cch + 1) * BCW])
                    nc.scalar.activation(
                        Wbc[:, cch * BCW:(cch + 1) * BCW],
                        bps[:, 0:BCW], AF.Copy, bias=0.0, scale=1.0)

                state[b] = (kT, h_T, a_T, Wbc)

            def phase2(b):
                kT, h_T, a_T, Wbc = state.pop(b)

                # t=0 cols of C: w[0, m] * Mv0[:, m]
                C = ccp.tile([128, WCOLS], f16, tag="C")
                t0w = _ap(Wbc[:], 0, [[L, M]])
                nc.vector.tensor_tensor(_ap(C[:], 0, [[L, M]]),
                                        c_Mv0[:], t0w, ALU.mult)

                fps = psF.tile([D, L], f32, tag="fps")

                for g in range(NGRP):
                    g0 = g * GCOLS
                    m0 = g * MGRP

                    # d0 = 1 - W*h  (reset cols: 0)
                    NW = unit.tile([128, SCOLS], f16, tag="NW")
                    nw_s = _ap(NW[:], 1, [[L + 1, MGRP], [1, L]])
                    wb_g = _ap(Wbc[:], g0, [[L, MGRP], [1, L]])
                    h_bc = _ap(h_T[:], 0, [[0, MGRP], [1, L]])
                    nc.vector.tensor_tensor(nw_s, wb_g, h_bc, ALU.mult)
                    nc.scalar.activation(nw_s, nw_s, AF.Copy,
                                         bias=1.0, scale=-1.0)
                    nc.vector.memset(_ap(NW[:], 0, [[L + 1, MGRP]]), 0.0)

                    # d1 = W*a  (reset cols: +Mv0 -> state resets to Mv0)
                    BN = unit.tile([128, SCOLS], f16, tag="BN")
                    bn_s = _ap(BN[:], 1, [[L + 1, MGRP], [1, L]])
                    a_bc = _ap(a_T[:], 0, [[0, MGRP], [1, L]])
                    nc.gpsimd.tensor_tensor(bn_s, wb_g, a_bc, ALU.mult)
                    nc.vector.tensor_copy(_ap(BN[:], 0, [[L + 1, MGRP]]),
                                          c_Mv0[:, m0:m0 + MGRP])

                    # scan: state = (d0 * state) + d1
                    Y = unit.tile([128, SCOLS], f16, tag="Y")
                    nc.vector.tensor_tensor_scan(Y[:], NW[:], BN[:], 0.0,
                                                 ALU.mult, ALU.add)

                    # G into C, m-major: C[m*L + t] = Y[m, t] * Wbc[m, t]
                    # (Y col j=t holds pre-update state for step t), t>=1
                    c_v = _ap(C[:], m0 * L + 1, [[L, MGRP], [1, L - 1]])
                    y_v = _ap(Y[:], 1, [[L + 1, MGRP], [1, L - 1]])
                    w_v = _ap(Wbc[:], g0 + 1, [[L, MGRP], [1, L - 1]])
                    nc.vector.tensor_tensor(c_v, y_v, w_v, ALU.mult)

                    # fps += sum_m fWr.T @ C_m for this group
                    for ml in range(MGRP):
                        m = m0 + ml
                        nc.tensor.matmul(
                            fps[:], c_fWrT[:],
                            _ap(C[:], m * L, [[1, L]]),
                            start=(m == 0), stop=False,
                            skip_group_check=True)

                nc.tensor.matmul(fps[:], c_fWkT[:], kT,
                                 start=False, stop=True,
                                 skip_group_check=True)
                f_T = sm.tile([D, L], f16, tag="f_T")
                nc.scalar.activation(f_T[:], fps[:], AF.Tanh,
                                     bias=c_fb[:], scale=1.0)
                pps = psP.tile([1, L], f32, tag="pps")
                nc.tensor.matmul(pps[:], c_pWT[:], f_T[:])
                th_p = sm.tile([1, L], f16, tag="th_p")
                nc.scalar.activation(th_p[:], pps[:], AF.Tanh,
                                     bias=c_pb2[:], scale=0.5)
                nc.vector.tensor_scalar(p_row[0:1, b * L:(b + 1) * L],
                                        th_p[:], 0.5, 0.5,
                                        ALU.mult, ALU.add)

            for b in range(BLn):
                phase1(b)
                if b >= 1:
                    phase2(b - 1)
            phase2(BLn - 1)

            nc.sync.dma_start(p_out[:, :],
                              _ap(p_row[:], 1, [[L, BLn], [1, L - 1]]))

    nc.compile()
    return nc


def _idx_table(ids):
    """ids [n, L] -> dma_gather idx table [128, n*NIDX/16] int16."""
    out = np.empty((128, ids.shape[0] * NIDX // 16), np.int16)
    for b in range(ids.shape[0]):
        pad = np.full(NIDX, -1, np.int16)
        pad[:L] = ids[b]
        tab = np.tile(pad.reshape(NIDX // 16, 16).T, (8, 1))
        out[:, b * (NIDX // 16):(b + 1) * (NIDX // 16)] = tab
    return out


def make_common(k_emb, v_emb, Mk, Mv0, e_W, e_b, a_W, a_b, f_W, f_b,
                p_W, p_b):
    return {
        "k_emb": np.asarray(k_emb, np.float16),
        "v_emb": np.asarray(v_emb, np.float16),
        "MkT": np.ascontiguousarray(np.asarray(Mk, np.float16).T),
        "eWT": np.ascontiguousarray(np.asarray(e_W, np.float16).T),
        "aWT": np.ascontiguousarray(np.asarray(a_W, np.float16).T),
        "fWrT": np.ascontiguousarray(np.asarray(f_W, np.float16)[:, :D].T),
        "fWkT": np.ascontiguousarray(np.asarray(f_W, np.float16)[:, D:].T),
        "pWT": np.ascontiguousarray(np.asarray(p_W, np.float16).T),
        "Mv0T16": np.ascontiguousarray(np.asarray(Mv0, np.float16).T),
        "ident": np.eye(D, dtype=np.float16),
        "ones16": np.ones((1, D), np.float16),
        "eb2": (np.asarray(e_b, np.float32) / 2).reshape(D, 1),
        "a_b": np.asarray(a_b, np.float32).reshape(D, 1),
        "f_b": np.asarray(f_b, np.float32).reshape(D, 1),
        "pb2": (np.asarray(p_b, np.float32) / 2).reshape(1, 1),
    }


def kernel(skills, responses, k_emb, v_emb, Mk, Mv0,
           e_W, e_b, a_W, a_b, f_W, f_b, p_W, p_b):
    skills = np.asarray(skills)
    responses = np.asarray(responses)

    masked_r = responses * (responses > -1).astype(responses.dtype)
    x = (skills.astype(np.int64) + NS * masked_r.astype(np.int64))

    common = make_common(k_emb, v_emb, Mk, Mv0, e_W, e_b, a_W, a_b,
                         f_W, f_b, p_W, p_b)

    in_maps = []
    for c in range(NCORES):
        bsl = slice(c * BL, (c + 1) * BL)
        m = dict(common)
        m["kidx"] = _idx_table(skills[bsl])
        m["vidx"] = _idx_table(x[bsl])
        in_maps.append(m)

    nc = build_bass()
    global LAST_RESULTS
    res = run_bass_kernel_spmd(nc, in_maps, core_ids=list(range(NCORES)),
                               trace=TRACE)
    LAST_RESULTS = res
    out = np.concatenate([res.results[c]["p_out"] for c in range(NCORES)],
                         axis=0)
    return out.astype(np.float32)


# revision 11
# speedup vs baseline: 1.1221x; 1.1221x over previous
"""DKVMN forward Trainium2 Bass kernel (fp16 bulk path, v2).

Model (per sample): embeddings -> softmax attention w over M memory slots ->
sequential memory update Mv_t = Mv_{t-1} * (1 - w_t e_t^T) + w_t a_t^T ->
weighted read of PRE-update memory -> output MLP -> sigmoid.

Sharding: data-parallel over batch. B=64 across 8 cores -> 8 samples/core.
Tables + weights replicated. Each core returns [8, 199]; host concatenates.

v2 structure (per core; engine-balanced against the TimelineSim cost model):
- natural-form scan (no sign trick): state = (d0 * state) + d1 with
  d0 = 1 - W*e, d1 = W*a; m-blocks chained in one scan instr per group via
  reset columns (d0=0, d1=Mv0 -> state resets to Mv0).
- sigmoid via tanh: sigmoid(x) = 0.5*tanh(x/2)+0.5 so every Act func
  (Tanh/Exp/Copy) lives in one act table -> no LoadActFuncSet swaps.
  The output affine folds into cheap DVE tensor_scalar (4x mode) ops.
- engine split: scans+NW+G on DVE, BN on Pool, 1-We affine + Wbc
  PSUM->SBUF copies on Act, broadcast + all matmuls incl. the 50-term
  m-reduction on PE (Ldweights free, PSUM accumulation).
- software pipelining: emit P1(b+1) (gather/emb/softmax/Wbc) before
  P2(b) (scan chain + readout) so each engine's in-order stream always
  has sample b+1 front-end work before sample b's back-end waits.
"""
import sys

sys.path.insert(0, "/opt/trn_rl_repo")

import numpy as np

import concourse.bacc as bacc
import concourse.bass as bass
import concourse.tile as tile
from concourse import library_config, mybir
from concourse.bass_utils import run_bass_kernel_spmd

f32 = mybir.dt.float32
f16 = mybir.dt.float16
i16 = mybir.dt.int16
AF = mybir.ActivationFunctionType
ALU = mybir.AluOpType
AX = mybir.AxisListType

B, L, NS, D, M = 64, 200, 1000, 128, 50
NCORES = 8
BL = B // NCORES          # samples per core
NIDX = 256                # padded gather idxs per sample (L=200 real; must be %128)
MGRP = 10                 # m's per scan group
NGRP = M // MGRP          # 5 groups
GCOLS = MGRP * L          # 2000 w-cols per group
SCOLS = MGRP * (L + 1)    # 2010 scan cols (incl. reset col per m)
WCOLS = M * L             # 10000
BCW = 500                 # broadcast chunk cols
NBC = WCOLS // BCW        # 20 chunks

TRACE = False
LAST_RESULTS = None


def _ap(t_ap, offset_add, free_dims):
    """Raw AP view: keep partition dim, replace free dims."""
    return bass.AP(t_ap.tensor, t_ap.offset + offset_add,
                   [t_ap.ap[0]] + free_dims)


def build_bass(n_samples=BL):
    BLn = n_samples
    nc = bacc.Bacc("TRN2", target_bir_lowering=False, debug=False,
                   num_devices=NCORES)

    def dram_in(name, shape, dtype=f32):
        return nc.dram_tensor(name, shape, dtype, kind="ExternalInput")

    k_emb = dram_in("k_emb", [NS, D], f16)
    v_emb = dram_in("v_emb", [2 * NS, D], f16)
    kidx = dram_in("kidx", [128, BLn * NIDX // 16], i16)
    vidx = dram_in("vidx", [128, BLn * NIDX // 16], i16)
    MkT = dram_in("MkT", [D, M], f16)
    eWT = dram_in("eWT", [D, D], f16)
    aWT = dram_in("aWT", [D, D], f16)
    fWrT = dram_in("fWrT", [D, D], f16)
    fWkT = dram_in("fWkT", [D, D], f16)
    pWT = dram_in("pWT", [D, 1], f16)
    Mv0T16 = dram_in("Mv0T16", [D, M], f16)
    ident = dram_in("ident", [D, D], f16)
    ones16 = dram_in("ones16", [1, D], f16)
    eb2 = dram_in("eb2", [D, 1])            # e_b / 2
    a_b = dram_in("a_b", [D, 1])
    f_b = dram_in("f_b", [D, 1])
    pb2 = dram_in("pb2", [1, 1])            # p_b / 2
    p_out = nc.dram_tensor("p_out", [BLn, L - 1], f32, kind="ExternalOutput")

    with tile.TileContext(nc) as tc:
        nc.gpsimd.load_library(library_config.mlp)
        with tc.tile_pool(name="const", bufs=1) as cpool, \
             tc.tile_pool(name="rows", bufs=3) as rpool, \
             tc.tile_pool(name="sm", bufs=2) as sm, \
             tc.tile_pool(name="wfp", bufs=2) as wfp, \
             tc.tile_pool(name="wbcp", bufs=2) as wbcp, \
             tc.tile_pool(name="unit", bufs=2) as unit, \
             tc.tile_pool(name="cc", bufs=3) as ccp, \
             tc.tile_pool(name="psE", bufs=1, space="PSUM") as psE, \
             tc.tile_pool(name="psA", bufs=1, space="PSUM") as psA, \
             tc.tile_pool(name="psF", bufs=2, space="PSUM") as psF, \
             tc.tile_pool(name="psP", bufs=1, space="PSUM") as psP, \
             tc.tile_pool(name="psW", bufs=1, space="PSUM") as psW, \
             tc.tile_pool(name="psBC", bufs=2, space="PSUM") as psBC:

            def cload(dram, shape, dtype=f32):
                t = cpool.tile(shape, dtype, tag=dram.name)
                nc.sync.dma_start(t[:], dram[(slice(None),) * len(shape)])
                return t

            c_MkT = cload(MkT, [D, M], f16)
            c_eWT = cload(eWT, [D, D], f16)
            c_aWT = cload(aWT, [D, D], f16)
            c_fWrT = cload(fWrT, [D, D], f16)
            c_fWkT = cload(fWkT, [D, D], f16)
            c_pWT = cload(pWT, [D, 1], f16)
            c_Mv0 = cload(Mv0T16, [D, M], f16)
            c_id = cload(ident, [D, D], f16)
            c_ones = cload(ones16, [1, D], f16)
            c_eb2 = cload(eb2, [D, 1])
            c_ab = cload(a_b, [D, 1])
            c_fb = cload(f_b, [D, 1])
            c_pb2 = cload(pb2, [1, 1])
            c_kidx = cload(kidx, [128, BLn * NIDX // 16], i16)
            c_vidx = cload(vidx, [128, BLn * NIDX // 16], i16)

            p_row = sm.tile([1, BLn * L], f32, tag="p_row")

            state = {}
            state2 = {}

            def phase1a(b):
                isl = slice(b * (NIDX // 16), (b + 1) * (NIDX // 16))

                # ---- transposed gathers: directly [D, t] fp16 ----
                kT3 = rpool.tile([128, 1, NIDX], f16, tag="kT3")
                nc.gpsimd.dma_gather(kT3[:], k_emb[:, :], c_kidx[:, isl],
                                     num_idxs=NIDX, num_idxs_reg=L,
                                     elem_size=D, transpose=True)
                vT3 = rpool.tile([128, 1, NIDX], f16, tag="vT3")
                nc.gpsimd.dma_gather(vT3[:], v_emb[:, :], c_vidx[:, isl],
                                     num_idxs=NIDX, num_idxs_reg=L,
                                     elem_size=D, transpose=True)
                kT = _ap(kT3[:], 0, [[1, L]])
                vT = _ap(vT3[:], 0, [[1, L]])

                # ---- h = sigmoid(e_W v + e_b) via tanh; a = tanh(...) ----
                eps = psE.tile([D, L], f32, tag="eps")
                nc.tensor.matmul(eps[:], c_eWT[:], vT)
                th_e = sm.tile([D, L], f16, tag="th_e")
                nc.scalar.activation(th_e[:], eps[:], AF.Tanh,
                                     bias=c_eb2[:], scale=0.5)
                h_T = sm.tile([D, L], f16, tag="h_T")
                nc.vector.tensor_scalar(h_T[:], th_e[:], 0.5, 0.5,
                                        ALU.mult, ALU.add)
                aps = psA.tile([D, L], f32, tag="aps")
                nc.tensor.matmul(aps[:], c_aWT[:], vT)
                a_T = sm.tile([D, L], f16, tag="a_T")
                nc.scalar.activation(a_T[:], aps[:], AF.Tanh,
                                     bias=c_ab[:], scale=1.0)

                # ---- w softmax (f32 psum) -> fp16 [m, t] ----
                wmT = sm.tile([M, L], f16, tag="wmT")
                for tb in range(2):
                    t0 = tb * 128
                    tsz = min(128, L - t0)
                    wps = psW.tile([128, M], f32, tag="wps")
                    nc.tensor.matmul(wps[0:tsz, :],
                                     _ap(kT3[:], t0, [[1, tsz]]),
                                     c_MkT[:])
                    negmax = sm.tile([128, 1], f32, tag="negmax")
                    nc.vector.tensor_reduce(negmax[0:tsz, :], wps[0:tsz, :],
                                            AX.X, ALU.max, negate=True)
                    wexp = sm.tile([128, M], f32, tag="wexp")
                    nc.scalar.activation(wexp[0:tsz, :], wps[0:tsz, :],
                                         AF.Exp, bias=negmax[0:tsz, :],
                                         scale=1.0)
                    ssum = sm.tile([128, 1], f32, tag="ssum")
                    nc.vector.tensor_reduce(ssum[0:tsz, :], wexp[0:tsz, :],
                                            AX.X, ALU.add)
                    rcp = sm.tile([128, 1], f32, tag="rcp")
                    nc.vector.reciprocal(rcp[0:tsz, :], ssum[0:tsz, :])
                    w16 = sm.tile([128, M], f16, tag="w16")
                    nc.vector.tensor_scalar_mul(w16[0:tsz, :], wexp[0:tsz, :],
                                                rcp[0:tsz, :])
                    wtp = psW.tile([M, 128], f16, tag="wps")
                    nc.tensor.transpose(wtp[:, 0:tsz], w16[0:tsz, :],
                                        c_id[0:tsz, 0:tsz])
                    nc.vector.tensor_copy(wmT[:, t0:t0 + tsz],
                                          wtp[:, 0:tsz])

                # ---- w_flat [1, M*L] fp16 (m-major) via DMA ----
                w_flat = wfp.tile([1, WCOLS], f16, tag="w_flat")
                nc.sync.dma_start(
                    _ap(w_flat[:], 0, [[L, M], [1, L]]), wmT[:])

                state[b] = (kT, h_T, a_T, w_flat)

            def phase1b(b):
                kT, h_T, a_T, w_flat = state.pop(b)
                # ---- Wbc broadcast via PE + ACT copies ----
                Wbc = wbcp.tile([128, WCOLS], f16, tag="Wbc")
                for cch in range(NBC):
                    bps = psBC.tile([128, BCW + 12], f32, tag="bc")
                    nc.tensor.matmul(
                        bps[:, 0:BCW], c_ones[:],
                        w_flat[0:1, cch * BCW:(cch + 1) * BCW])
                    nc.scalar.activation(
                        Wbc[:, cch * BCW:(cch + 1) * BCW],
                        bps[:, 0:BCW], AF.Copy, bias=0.0, scale=1.0)
                state2[b] = (kT, h_T, a_T, Wbc)

            def phase2a(b):
                kT, h_T, a_T, Wbc = state2.pop(b)

                # t=0 cols of C: w[0, m] * Mv0[:, m]
                C = ccp.tile([128, WCOLS], f16, tag="C")
                t0w = _ap(Wbc[:], 0, [[L, M]])
                nc.vector.tensor_tensor(_ap(C[:], 0, [[L, M]]),
                                        c_Mv0[:], t0w, ALU.mult)

                for g in range(NGRP):
                    g0 = g * GCOLS
                    m0 = g * MGRP

                    # d0 = 1 - W*h  (reset cols: 0)
                    NW = unit.tile([128, SCOLS], f16, tag="NW")
                    nw_s = _ap(NW[:], 1, [[L + 1, MGRP], [1, L]])
                    wb_g = _ap(Wbc[:], g0, [[L, MGRP], [1, L]])
                    h_bc = _ap(h_T[:], 0, [[0, MGRP], [1, L]])
                    nc.vector.tensor_tensor(nw_s, wb_g, h_bc, ALU.mult)
                    nc.scalar.activation(nw_s, nw_s, AF.Copy,
                                         bias=1.0, scale=-1.0)
                    nc.vector.memset(_ap(NW[:], 0, [[L + 1, MGRP]]), 0.0)

                    # d1 = W*a  (reset cols: +Mv0 -> state resets to Mv0)
                    BN = unit.tile([128, SCOLS], f16, tag="BN")
                    bn_s = _ap(BN[:], 1, [[L + 1, MGRP], [1, L]])
                    a_bc = _ap(a_T[:], 0, [[0, MGRP], [1, L]])
                    nc.gpsimd.tensor_tensor(bn_s, wb_g, a_bc, ALU.mult)
                    nc.vector.tensor_copy(_ap(BN[:], 0, [[L + 1, MGRP]]),
                                          c_Mv0[:, m0:m0 + MGRP])

                    # scan: state = (d0 * state) + d1
                    Y = unit.tile([128, SCOLS], f16, tag="Y")
                    nc.vector.tensor_tensor_scan(Y[:], NW[:], BN[:], 0.0,
                                                 ALU.mult, ALU.add)

                    # G into C, m-major: C[m*L + t] = Y[m, t] * Wbc[m, t]
                    # (Y col j=t holds pre-update state for step t), t>=1
                    c_v = _ap(C[:], m0 * L + 1, [[L, MGRP], [1, L - 1]])
                    y_v = _ap(Y[:], 1, [[L + 1, MGRP], [1, L - 1]])
                    w_v = _ap(Wbc[:], g0 + 1, [[L, MGRP], [1, L - 1]])
                    nc.vector.tensor_tensor(c_v, y_v, w_v, ALU.mult)

                state2[b] = (kT, C)

            def phase2b(b):
                kT, C = state2.pop(b)
                fps = psF.tile([D, L], f32, tag="fps")
                for m in range(M):
                    nc.tensor.matmul(
                        fps[:], c_fWrT[:],
                        _ap(C[:], m * L, [[1, L]]),
                        start=(m == 0), stop=False,
                        skip_group_check=True)
                nc.tensor.matmul(fps[:], c_fWkT[:], kT,
                                 start=False, stop=True,
                                 skip_group_check=True)
                f_T = sm.tile([D, L], f16, tag="f_T")
                nc.scalar.activation(f_T[:], fps[:], AF.Tanh,
                                     bias=c_fb[:], scale=1.0)
                pps = psP.tile([1, L], f32, tag="pps")
                nc.tensor.matmul(pps[:], c_pWT[:], f_T[:])
                th_p = sm.tile([1, L], f16, tag="th_p")
                nc.scalar.activation(th_p[:], pps[:], AF.Tanh,
                                     bias=c_pb2[:], scale=0.5)
                nc.vector.tensor_scalar(p_row[0:1, b * L:(b + 1) * L],
                                        th_p[:], 0.5, 0.5,
                                        ALU.mult, ALU.add)

            for i in range(BLn + 2):
                if i < BLn:
                    phase1a(i)
                if 1 <= i <= BLn:
                    phase2a(i - 1)
                if i < BLn:
                    phase1b(i)
                if i >= 2:
                    phase2b(i - 2)

            nc.sync.dma_start(p_out[:, :],
                              _ap(p_row[:], 1, [[L, BLn], [1, L - 1]]))

    nc.compile()
    return nc


def _idx_table(ids):
    """ids [n, L] -> dma_gather idx table [128, n*NIDX/16] int16."""
    out = np.empty((128, ids.shape[0] * NIDX // 16), np.int16)
    for b in range(ids.shape[0]):
        pad = np.full(NIDX, -1, np.int16)
        pad[:L] = ids[b]
        tab = np.tile(pad.reshape(NIDX // 16, 16).T, (8, 1))
        out[:, b * (NIDX // 16):(b + 1) * (NIDX // 16)] = tab
    return out


def make_common(k_emb, v_emb, Mk, Mv0, e_W, e_b, a_W, a_b, f_W, f_b,
                p_W, p_b):
    return {
        "k_emb": np.asarray(k_emb, np.float16),
        "v_emb": np.asarray(v_emb, np.float16),
        "MkT": np.ascontiguousarray(np.asarray(Mk, np.float16).T),
        "eWT": np.ascontiguousarray(np.asarray(e_W, np.float16).T),
        "aWT": np.ascontiguousarray(np.asarray(a_W, np.float16).T),
        "fWrT": np.ascontiguousarray(np.asarray(f_W, np.float16)[:, :D].T),
        "fWkT": np.ascontiguousarray(np.asarray(f_W, np.float16)[:, D:].T),
        "pWT": np.ascontiguousarray(np.asarray(p_W, np.float16).T),
        "Mv0T16": np.ascontiguousarray(np.asarray(Mv0, np.float16).T),
        "ident": np.eye(D, dtype=np.float16),
        "ones16": np.ones((1, D), np.float16),
        "eb2": (np.asarray(e_b, np.float32) / 2).reshape(D, 1),
        "a_b": np.asarray(a_b, np.float32).reshape(D, 1),
        "f_b": np.asarray(f_b, np.float32).reshape(D, 1),
        "pb2": (np.asarray(p_b, np.float32) / 2).reshape(1, 1),
    }


def kernel(skills, responses, k_emb, v_emb, Mk, Mv0,
           e_W, e_b, a_W, a_b, f_W, f_b, p_W, p_b):
    skills = np.asarray(skills)
    responses = np.asarray(responses)

    masked_r = responses * (responses > -1).astype(responses.dtype)
    x = (skills.astype(np.int64) + NS * masked_r.astype(np.int64))

    common = make_common(k_emb, v_emb, Mk, Mv0, e_W, e_b, a_W, a_b,
                         f_W, f_b, p_W, p_b)

    in_maps = []
    for c in range(NCORES):
        bsl = slice(c * BL, (c + 1) * BL)
        m = dict(common)
        m["kidx"] = _idx_table(skills[bsl])
        m["vidx"] = _idx_table(x[bsl])
        in_maps.append(m)

    nc = build_bass()
    global LAST_RESULTS
    res = run_bass_kernel_spmd(nc, in_maps, core_ids=list(range(NCORES)),
                               trace=TRACE)
    LAST_RESULTS = res
    out = np.concatenate([res.results[c]["p_out"] for c in range(NCORES)],
                         axis=0)
    return out.astype(np.float32)


# revision 14
# speedup vs baseline: 1.1272x; 1.0045x over previous
"""DKVMN forward Trainium2 Bass kernel (fp16 bulk path, v2).

Model (per sample): embeddings -> softmax attention w over M memory slots ->
sequential memory update Mv_t = Mv_{t-1} * (1 - w_t e_t^T) + w_t a_t^T ->
weighted read of PRE-update memory -> output MLP -> sigmoid.

Sharding: data-parallel over batch. B=64 across 8 cores -> 8 samples/core.
Tables + weights replicated. Each core returns [8, 199]; host concatenates.

v2 structure (per core; engine-balanced against the TimelineSim cost model):
- natural-form scan (no sign trick): state = (d0 * state) + d1 with
  d0 = 1 - W*e, d1 = W*a; m-blocks chained in one scan instr per group via
  reset columns (d0=0, d1=Mv0 -> state resets to Mv0).
- sigmoid via tanh: sigmoid(x) = 0.5*tanh(x/2)+0.5 so every Act func
  (Tanh/Exp/Copy) lives in one act table -> no LoadActFuncSet swaps.
  The output affine folds into cheap DVE tensor_scalar (4x mode) ops.
- engine split: scans+NW+G on DVE, BN on Pool, 1-We affine + Wbc
  PSUM->SBUF copies on Act, broadcast + all matmuls incl. the 50-term
  m-reduction on PE (Ldweights free, PSUM accumulation).
- software pipelining: emit P1(b+1) (gather/emb/softmax/Wbc) before
  P2(b) (scan chain + readout) so each engine's in-order stream always
  has sample b+1 front-end work before sample b's back-end waits.
"""
import sys

sys.path.insert(0, "/opt/trn_rl_repo")

import numpy as np

import concourse.bacc as bacc
import concourse.bass as bass
import concourse.tile as tile
from concourse import library_config, mybir
from concourse.bass_utils import run_bass_kernel_spmd

f32 = mybir.dt.float32
f16 = mybir.dt.float16
i16 = mybir.dt.int16
AF = mybir.ActivationFunctionType
ALU = mybir.AluOpType
AX = mybir.AxisListType

B, L, NS, D, M = 64, 200, 1000, 128, 50
NCORES = 8
BL = B // NCORES          # samples per core
NIDX = 256                # padded gather idxs per sample (L=200 real; must be %128)
MGRP = 10                 # m's per scan group
NGRP = M // MGRP          # 5 groups
GCOLS = MGRP * L          # 2000 w-cols per group
SCOLS = MGRP * (L + 1)    # 2010 scan cols (incl. reset col per m)
WCOLS = M * L             # 10000

TRACE = False
LAST_RESULTS = None


def _ap(t_ap, offset_add, free_dims):
    """Raw AP view: keep partition dim, replace free dims."""
    return bass.AP(t_ap.tensor, t_ap.offset + offset_add,
                   [t_ap.ap[0]] + free_dims)


def build_bass(n_samples=BL):
    BLn = n_samples
    nc = bacc.Bacc("TRN2", target_bir_lowering=False, debug=False,
                   num_devices=NCORES)

    def dram_in(name, shape, dtype=f32):
        return nc.dram_tensor(name, shape, dtype, kind="ExternalInput")

    k_emb = dram_in("k_emb", [NS, D], f16)
    v_emb = dram_in("v_emb", [2 * NS, D], f16)
    kidx = dram_in("kidx", [128, BLn * NIDX // 16], i16)
    vidx = dram_in("vidx", [128, BLn * NIDX // 16], i16)
    MkT = dram_in("MkT", [D, M], f16)
    eWT = dram_in("eWT", [D, D], f16)
    aWT = dram_in("aWT", [D, D], f16)
    fWrT = dram_in("fWrT", [D, D], f16)
    fWkT = dram_in("fWkT", [D, D], f16)
    pWT = dram_in("pWT", [D, 1], f16)
    Mv0T16 = dram_in("Mv0T16", [D, M], f16)
    ident = dram_in("ident", [D, D], f16)
    eb2 = dram_in("eb2", [D, 1])            # e_b / 2
    a_b = dram_in("a_b", [D, 1])
    f_b = dram_in("f_b", [D, 1])
    pb2 = dram_in("pb2", [1, 1])            # p_b / 2
    p_out = nc.dram_tensor("p_out", [BLn, L - 1], f32, kind="ExternalOutput")

    with tile.TileContext(nc) as tc:
        nc.gpsimd.load_library(library_config.mlp)
        with tc.tile_pool(name="const", bufs=1) as cpool, \
             tc.tile_pool(name="rows", bufs=3) as rpool, \
             tc.tile_pool(name="sm", bufs=2) as sm, \
             tc.tile_pool(name="wfp", bufs=2) as wfp, \
             tc.tile_pool(name="wbcp", bufs=2) as wbcp, \
             tc.tile_pool(name="unit", bufs=2) as unit, \
             tc.tile_pool(name="cc", bufs=2) as ccp, \
             tc.tile_pool(name="psE", bufs=1, space="PSUM") as psE, \
             tc.tile_pool(name="psA", bufs=1, space="PSUM") as psA, \
             tc.tile_pool(name="psF", bufs=2, space="PSUM") as psF, \
             tc.tile_pool(name="psP", bufs=1, space="PSUM") as psP, \
             tc.tile_pool(name="psW", bufs=1, space="PSUM") as psW:

            def cload(dram, shape, dtype=f32):
                t = cpool.tile(shape, dtype, tag=dram.name)
                nc.sync.dma_start(t[:], dram[(slice(None),) * len(shape)])
                return t

            c_MkT = cload(MkT, [D, M], f16)
            c_eWT = cload(eWT, [D, D], f16)
            c_aWT = cload(aWT, [D, D], f16)
            c_fWrT = cload(fWrT, [D, D], f16)
            c_fWkT = cload(fWkT, [D, D], f16)
            c_pWT = cload(pWT, [D, 1], f16)
            c_Mv0 = cload(Mv0T16, [D, M], f16)
            c_id = cload(ident, [D, D], f16)
            c_eb2 = cload(eb2, [D, 1])
            c_ab = cload(a_b, [D, 1])
            c_fb = cload(f_b, [D, 1])
            c_pb2 = cload(pb2, [1, 1])
            c_kidx = cload(kidx, [128, BLn * NIDX // 16], i16)
            c_vidx = cload(vidx, [128, BLn * NIDX // 16], i16)

            p_row = sm.tile([1, BLn * L], f32, tag="p_row")

            state = {}
            state2 = {}

            def phase1a(b):
                isl = slice(b * (NIDX // 16), (b + 1) * (NIDX // 16))

                # ---- transposed gathers: directly [D, t] fp16 ----
                kT3 = rpool.tile([128, 1, NIDX], f16, tag="kT3")
                nc.gpsimd.dma_gather(kT3[:], k_emb[:, :], c_kidx[:, isl],
                                     num_idxs=NIDX, num_idxs_reg=L,
                                     elem_size=D, transpose=True)
                vT3 = rpool.tile([128, 1, NIDX], f16, tag="vT3")
                nc.gpsimd.dma_gather(vT3[:], v_emb[:, :], c_vidx[:, isl],
                                     num_idxs=NIDX, num_idxs_reg=L,
                                     elem_size=D, transpose=True)
                kT = _ap(kT3[:], 0, [[1, L]])
                vT = _ap(vT3[:], 0, [[1, L]])

                # ---- h = sigmoid(e_W v + e_b) via tanh; a = tanh(...) ----
                eps = psE.tile([D, L], f32, tag="eps")
                nc.tensor.matmul(eps[:], c_eWT[:], vT)
                th_e = sm.tile([D, L], f16, tag="th_e")
                nc.scalar.activation(th_e[:], eps[:], AF.Tanh,
                                     bias=c_eb2[:], scale=0.5)
                h_T = sm.tile([D, L], f16, tag="h_T")
                nc.vector.tensor_scalar(h_T[:], th_e[:], 0.5, 0.5,
                                        ALU.mult, ALU.add)
                aps = psA.tile([D, L], f32, tag="aps")
                nc.tensor.matmul(aps[:], c_aWT[:], vT)
                a_T = sm.tile([D, L], f16, tag="a_T")
                nc.scalar.activation(a_T[:], aps[:], AF.Tanh,
                                     bias=c_ab[:], scale=1.0)

                # ---- w softmax (f32 psum) -> fp16 [m, t] ----
                wmT = sm.tile([M, L], f16, tag="wmT")
                for tb in range(2):
                    t0 = tb * 128
                    tsz = min(128, L - t0)
                    wps = psW.tile([128, M], f32, tag="wps")
                    nc.tensor.matmul(wps[0:tsz, :],
                                     _ap(kT3[:], t0, [[1, tsz]]),
                                     c_MkT[:])
                    negmax = sm.tile([128, 1], f32, tag="negmax")
                    nc.vector.tensor_reduce(negmax[0:tsz, :], wps[0:tsz, :],
                                            AX.X, ALU.max, negate=True)
                    wexp = sm.tile([128, M], f32, tag="wexp")
                    nc.scalar.activation(wexp[0:tsz, :], wps[0:tsz, :],
                                         AF.Exp, bias=negmax[0:tsz, :],
                                         scale=1.0)
                    ssum = sm.tile([128, 1], f32, tag="ssum")
                    nc.vector.tensor_reduce(ssum[0:tsz, :], wexp[0:tsz, :],
                                            AX.X, ALU.add)
                    rcp = sm.tile([128, 1], f32, tag="rcp")
                    nc.vector.reciprocal(rcp[0:tsz, :], ssum[0:tsz, :])
                    w16 = sm.tile([128, M], f16, tag="w16")
                    nc.vector.tensor_scalar_mul(w16[0:tsz, :], wexp[0:tsz, :],
                                                rcp[0:tsz, :])
                    wtp = psW.tile([M, 128], f16, tag="wps")
                    nc.tensor.transpose(wtp[:, 0:tsz], w16[0:tsz, :],
                                        c_id[0:tsz, 0:tsz])
                    nc.vector.tensor_copy(wmT[:, t0:t0 + tsz],
                                          wtp[:, 0:tsz])

                # ---- w_flat [1, M*L] fp16 (m-major) via DMA, then DMA
                # partition-broadcast to all 128 partitions ----
                w_flat = wfp.tile([1, WCOLS], f16, tag="w_flat")
                nc.sync.dma_start(
                    _ap(w_flat[:], 0, [[L, M], [1, L]]), wmT[:])
                Wbc = wbcp.tile([128, WCOLS], f16, tag="Wbc")
                wf_bc = bass.AP(w_flat[:].tensor, w_flat[:].offset,
                                [[1, 1], [0, 128], [1, WCOLS]])
                nc.sync.dma_start(Wbc[:], wf_bc)

                state[b] = (kT, h_T, a_T, Wbc)

            def phase2a(b):
                kT, h_T, a_T, Wbc = state.pop(b)

                # t=0 cols of C: w[0, m] * Mv0[:, m]
                C = ccp.tile([128, WCOLS], f16, tag="C")
                t0w = _ap(Wbc[:], 0, [[L, M]])
                nc.vector.tensor_tensor(_ap(C[:], 0, [[L, M]]),
                                        c_Mv0[:], t0w, ALU.mult)

                for g in range(NGRP):
                    g0 = g * GCOLS
                    m0 = g * MGRP

                    # d0 = 1 - W*h  (reset cols: 0)
                    NW = unit.tile([128, SCOLS], f16, tag="NW")
                    nw_s = _ap(NW[:], 1, [[L + 1, MGRP], [1, L]])
                    wb_g = _ap(Wbc[:], g0, [[L, MGRP], [1, L]])
                    h_bc = _ap(h_T[:], 0, [[0, MGRP], [1, L]])
                    nc.vector.tensor_tensor(nw_s, wb_g, h_bc, ALU.mult)
                    nc.scalar.activation(nw_s, nw_s, AF.Copy,
                                         bias=1.0, scale=-1.0)
                    nc.vector.memset(_ap(NW[:], 0, [[L + 1, MGRP]]), 0.0)

                    # d1 = W*a  (reset cols: +Mv0 -> state resets to Mv0)
                    BN = unit.tile([128, SCOLS], f16, tag="BN")
                    bn_s = _ap(BN[:], 1, [[L + 1, MGRP], [1, L]])
                    a_bc = _ap(a_T[:], 0, [[0, MGRP], [1, L]])
                    nc.gpsimd.tensor_tensor(bn_s, wb_g, a_bc, ALU.mult)
                    nc.vector.tensor_copy(_ap(BN[:], 0, [[L + 1, MGRP]]),
                                          c_Mv0[:, m0:m0 + MGRP])

                    # scan: state = (d0 * state) + d1
                    Y = unit.tile([128, SCOLS], f16, tag="Y")
                    nc.vector.tensor_tensor_scan(Y[:], NW[:], BN[:], 0.0,
                                                 ALU.mult, ALU.add)

                    # G into C, m-major: C[m*L + t] = Y[m, t] * Wbc[m, t]
                    # (Y col j=t holds pre-update state for step t), t>=1
                    c_v = _ap(C[:], m0 * L + 1, [[L, MGRP], [1, L - 1]])
                    y_v = _ap(Y[:], 1, [[L + 1, MGRP], [1, L - 1]])
                    w_v = _ap(Wbc[:], g0 + 1, [[L, MGRP], [1, L - 1]])
                    nc.vector.tensor_tensor(c_v, y_v, w_v, ALU.mult)

                state2[b] = (kT, C)

            def phase2b(b):
                kT, C = state2.pop(b)
                fps = psF.tile([D, L], f32, tag="fps")
                for m in range(M):
                    nc.tensor.matmul(
                        fps[:], c_fWrT[:],
                        _ap(C[:], m * L, [[1, L]]),
                        start=(m == 0), stop=False,
                        skip_group_check=True)
                nc.tensor.matmul(fps[:], c_fWkT[:], kT,
                                 start=False, stop=True,
                                 skip_group_check=True)
                f_T = sm.tile([D, L], f16, tag="f_T")
                nc.scalar.activation(f_T[:], fps[:], AF.Tanh,
                                     bias=c_fb[:], scale=1.0)
                pps = psP.tile([1, L], f32, tag="pps")
                nc.tensor.matmul(pps[:], c_pWT[:], f_T[:])
                th_p = sm.tile([1, L], f16, tag="th_p")
                nc.scalar.activation(th_p[:], pps[:], AF.Tanh,
                                     bias=c_pb2[:], scale=0.5)
                nc.vector.tensor_scalar(p_row[0:1, b * L:(b + 1) * L],
                                        th_p[:], 0.5, 0.5,
                                        ALU.mult, ALU.add)

            for i in range(BLn + 2):
                if i < BLn:
                    phase1a(i)
                if 1 <= i <= BLn:
                    phase2a(i - 1)
                if i >= 2:
                    phase2b(i - 2)

            nc.sync.dma_start(p_out[:, :],
                              _ap(p_row[:], 1, [[L, BLn], [1, L - 1]]))

    nc.compile()
    return nc


def _idx_table(ids):
    """ids [n, L] -> dma_gather idx table [128, n*NIDX/16] int16."""
    out = np.empty((128, ids.shape[0] * NIDX // 16), np.int16)
    for b in range(ids.shape[0]):
        pad = np.full(NIDX, -1, np.int16)
        pad[:L] = ids[b]
        tab = np.tile(pad.reshape(NIDX // 16, 16).T, (8, 1))
        out[:, b * (NIDX // 16):(b + 1) * (NIDX // 16)] = tab
    return out


def make_common(k_emb, v_emb, Mk, Mv0, e_W, e_b, a_W, a_b, f_W, f_b,
                p_W, p_b):
    return {
        "k_emb": np.asarray(k_emb, np.float16),
        "v_emb": np.asarray(v_emb, np.float16),
        "MkT": np.ascontiguousarray(np.asarray(Mk, np.float16).T),
        "eWT": np.ascontiguousarray(np.asarray(e_W, np.float16).T),
        "aWT": np.ascontiguousarray(np.asarray(a_W, np.float16).T),
        "fWrT": np.ascontiguousarray(np.asarray(f_W, np.float16)[:, :D].T),
        "fWkT": np.ascontiguousarray(np.asarray(f_W, np.float16)[:, D:].T),
        "pWT": np.ascontiguousarray(np.asarray(p_W, np.float16).T),
        "Mv0T16": np.ascontiguousarray(np.asarray(Mv0, np.float16).T),
        "ident": np.eye(D, dtype=np.float16),
        "eb2": (np.asarray(e_b, np.float32) / 2).reshape(D, 1),
        "a_b": np.asarray(a_b, np.float32).reshape(D, 1),
        "f_b": np.asarray(f_b, np.float32).reshape(D, 1),
        "pb2": (np.asarray(p_b, np.float32) / 2).reshape(1, 1),
    }


def kernel(skills, responses, k_emb, v_emb, Mk, Mv0,
           e_W, e_b, a_W, a_b, f_W, f_b, p_W, p_b):
    skills = np.asarray(skills)
    responses = np.asarray(responses)

    masked_r = responses * (responses > -1).astype(responses.dtype)
    x = (skills.astype(np.int64) + NS * masked_r.astype(np.int64))

    common = make_common(k_emb, v_emb, Mk, Mv0, e_W, e_b, a_W, a_b,
                         f_W, f_b, p_W, p_b)

    in_maps = []
    for c in range(NCORES):
        bsl = slice(c * BL, (c + 1) * BL)
        m = dict(common)
        m["kidx"] = _idx_table(skills[bsl])
        m["vidx"] = _idx_table(x[bsl])
        in_maps.append(m)

    nc = build_bass()
    global LAST_RESULTS
    res = run_bass_kernel_spmd(nc, in_maps, core_ids=list(range(NCORES)),
                               trace=TRACE)
    LAST_RESULTS = res
    out = np.concatenate([res.results[c]["p_out"] for c in range(NCORES)],
                         axis=0)
    return out.astype(np.float32)


# revision 18
# speedup vs baseline: 1.2328x; 1.0937x over previous
"""DKVMN forward Trainium2 Bass kernel (fp16 bulk path, v2).

Model (per sample): embeddings -> softmax attention w over M memory slots ->
sequential memory update Mv_t = Mv_{t-1} * (1 - w_t e_t^T) + w_t a_t^T ->
weighted read of PRE-update memory -> output MLP -> sigmoid.

Sharding: data-parallel over batch. B=64 across 8 cores -> 8 samples/core.
Tables + weights replicated. Each core returns [8, 199]; host concatenates.

v2 structure (per core; engine-balanced against the TimelineSim cost model):
- natural-form scan (no sign trick): state = (d0 * state) + d1 with
  d0 = 1 - W*e, d1 = W*a; m-blocks chained in one scan instr per group via
  reset columns (d0=0, d1=Mv0 -> state resets to Mv0).
- sigmoid via tanh: sigmoid(x) = 0.5*tanh(x/2)+0.5 so every Act func
  (Tanh/Exp/Copy) lives in one act table -> no LoadActFuncSet swaps.
  The output affine folds into cheap DVE tensor_scalar (4x mode) ops.
- engine split: scans+NW+G on DVE, BN on Pool, 1-We affine + Wbc
  PSUM->SBUF copies on Act, broadcast + all matmuls incl. the 50-term
  m-reduction on PE (Ldweights free, PSUM accumulation).
- software pipelining: emit P1(b+1) (gather/emb/softmax/Wbc) before
  P2(b) (scan chain + readout) so each engine's in-order stream always
  has sample b+1 front-end work before sample b's back-end waits.
"""
import sys

sys.path.insert(0, "/opt/trn_rl_repo")

import numpy as np

import concourse.bacc as bacc
import concourse.bass as bass
import concourse.tile as tile
from concourse import library_config, mybir
from concourse.bass_utils import run_bass_kernel_spmd

f32 = mybir.dt.float32
f16 = mybir.dt.float16
i16 = mybir.dt.int16
AF = mybir.ActivationFunctionType
ALU = mybir.AluOpType
AX = mybir.AxisListType

B, L, NS, D, M = 64, 200, 1000, 128, 50
NCORES = 8
BL = B // NCORES          # samples per core
NIDX = 256                # padded gather idxs per sample (L=200 real; must be %128)
MGRP = 10                 # m's per scan group
NGRP = M // MGRP          # 5 groups
GCOLS = MGRP * L          # 2000 w-cols per group
SCOLS = MGRP * (L + 1)    # 2010 scan cols (incl. reset col per m)
WCOLS = M * L             # 10000

TRACE = False
LAST_RESULTS = None


def _ap(t_ap, offset_add, free_dims):
    """Raw AP view: keep partition dim, replace free dims."""
    return bass.AP(t_ap.tensor, t_ap.offset + offset_add,
                   [t_ap.ap[0]] + free_dims)


def build_bass(n_samples=BL):
    BLn = n_samples
    nc = bacc.Bacc("TRN2", target_bir_lowering=False, debug=False,
                   num_devices=NCORES)

    def dram_in(name, shape, dtype=f32):
        return nc.dram_tensor(name, shape, dtype, kind="ExternalInput")

    k_emb = dram_in("k_emb", [NS, D], f16)
    v_emb = dram_in("v_emb", [2 * NS, D], f16)
    kidx = dram_in("kidx", [128, BLn * NIDX // 16], i16)
    vidx = dram_in("vidx", [128, BLn * NIDX // 16], i16)
    MkT = dram_in("MkT", [D, M], f16)
    eWT = dram_in("eWT", [D, D], f16)
    aWT = dram_in("aWT", [D, D], f16)
    fWrT = dram_in("fWrT", [D, D], f16)
    fWkT = dram_in("fWkT", [D, D], f16)
    pWT = dram_in("pWT", [D, 1], f16)
    Mv0T16 = dram_in("Mv0T16", [D, M], f16)
    ident = dram_in("ident", [D, D], f16)
    eb2 = dram_in("eb2", [D, 1])            # e_b / 2
    a_b = dram_in("a_b", [D, 1])
    f_b = dram_in("f_b", [D, 1])
    pb2 = dram_in("pb2", [1, 1])            # p_b / 2
    p_out = nc.dram_tensor("p_out", [BLn, L - 1], f32, kind="ExternalOutput")

    with tile.TileContext(nc) as tc:
        nc.gpsimd.load_library(library_config.mlp)
        with tc.tile_pool(name="const", bufs=1) as cpool, \
             tc.tile_pool(name="rows", bufs=6) as rpool, \
             tc.tile_pool(name="sm", bufs=2) as sm, \
             tc.tile_pool(name="wfp", bufs=2) as wfp, \
             tc.tile_pool(name="wbcp", bufs=2) as wbcp, \
             tc.tile_pool(name="unit", bufs=2) as unit, \
             tc.tile_pool(name="cc", bufs=2) as ccp, \
             tc.tile_pool(name="psE", bufs=1, space="PSUM") as psE, \
             tc.tile_pool(name="psA", bufs=1, space="PSUM") as psA, \
             tc.tile_pool(name="psF", bufs=2, space="PSUM") as psF, \
             tc.tile_pool(name="psP", bufs=1, space="PSUM") as psP, \
             tc.tile_pool(name="psW", bufs=1, space="PSUM") as psW:

            def cload(dram, shape, dtype=f32):
                t = cpool.tile(shape, dtype, tag=dram.name)
                nc.sync.dma_start(t[:], dram[(slice(None),) * len(shape)])
                return t

            c_MkT = cload(MkT, [D, M], f16)
            c_eWT = cload(eWT, [D, D], f16)
            c_aWT = cload(aWT, [D, D], f16)
            c_fWrT = cload(fWrT, [D, D], f16)
            c_fWkT = cload(fWkT, [D, D], f16)
            c_pWT = cload(pWT, [D, 1], f16)
            c_Mv0 = cload(Mv0T16, [D, M], f16)
            c_id = cload(ident, [D, D], f16)
            c_eb2 = cload(eb2, [D, 1])
            c_ab = cload(a_b, [D, 1])
            c_fb = cload(f_b, [D, 1])
            c_pb2 = cload(pb2, [1, 1])
            c_kidx = cload(kidx, [128, BLn * NIDX // 16], i16)
            c_vidx = cload(vidx, [128, BLn * NIDX // 16], i16)

            p_row = sm.tile([1, BLn * L], f32, tag="p_row")

            state = {}
            state2 = {}

            gat = {}

            def gathers(b):
                isl = slice(b * (NIDX // 16), (b + 1) * (NIDX // 16))
                # ---- transposed gathers: directly [D, t] fp16 ----
                kT3 = rpool.tile([128, 1, NIDX], f16, tag="kT3")
                nc.gpsimd.dma_gather(kT3[:], k_emb[:, :], c_kidx[:, isl],
                                     num_idxs=NIDX, num_idxs_reg=L,
                                     elem_size=D, transpose=True)
                vT3 = rpool.tile([128, 1, NIDX], f16, tag="vT3")
                nc.gpsimd.dma_gather(vT3[:], v_emb[:, :], c_vidx[:, isl],
                                     num_idxs=NIDX, num_idxs_reg=L,
                                     elem_size=D, transpose=True)
                gat[b] = (kT3, vT3)

            def phase1a(b):
                kT3, vT3 = gat.pop(b)
                kT = _ap(kT3[:], 0, [[1, L]])
                vT = _ap(vT3[:], 0, [[1, L]])

                # ---- h = sigmoid(e_W v + e_b) via tanh; a = tanh(...) ----
                eps = psE.tile([D, L], f32, tag="eps")
                nc.tensor.matmul(eps[:], c_eWT[:], vT)
                th_e = sm.tile([D, L], f16, tag="th_e")
                nc.scalar.activation(th_e[:], eps[:], AF.Tanh,
                                     bias=c_eb2[:], scale=0.5)
                h_T = sm.tile([D, L], f16, tag="h_T")
                nc.vector.tensor_scalar(h_T[:], th_e[:], 0.5, 0.5,
                                        ALU.mult, ALU.add)
                aps = psA.tile([D, L], f32, tag="aps")
                nc.tensor.matmul(aps[:], c_aWT[:], vT)
                a_T = sm.tile([D, L], f16, tag="a_T")
                nc.scalar.activation(a_T[:], aps[:], AF.Tanh,
                                     bias=c_ab[:], scale=1.0)

                # ---- w softmax (f32 psum) -> fp16 [m, t] ----
                wmT = sm.tile([M, L], f16, tag="wmT")
                for tb in range(2):
                    t0 = tb * 128
                    tsz = min(128, L - t0)
                    wps = psW.tile([128, M], f32, tag="wps")
                    nc.tensor.matmul(wps[0:tsz, :],
                                     _ap(kT3[:], t0, [[1, tsz]]),
                                     c_MkT[:])
                    negmax = sm.tile([128, 1], f32, tag="negmax")
                    nc.vector.tensor_reduce(negmax[0:tsz, :], wps[0:tsz, :],
                                            AX.X, ALU.max, negate=True)
                    wexp = sm.tile([128, M], f32, tag="wexp")
                    nc.scalar.activation(wexp[0:tsz, :], wps[0:tsz, :],
                                         AF.Exp, bias=negmax[0:tsz, :],
                                         scale=1.0)
                    ssum = sm.tile([128, 1], f32, tag="ssum")
                    nc.vector.tensor_reduce(ssum[0:tsz, :], wexp[0:tsz, :],
                                            AX.X, ALU.add)
                    rcp = sm.tile([128, 1], f32, tag="rcp")
                    nc.vector.reciprocal(rcp[0:tsz, :], ssum[0:tsz, :])
                    w16 = sm.tile([128, M], f16, tag="w16")
                    nc.vector.tensor_scalar_mul(w16[0:tsz, :], wexp[0:tsz, :],
                                                rcp[0:tsz, :])
                    wtp = psW.tile([M, 128], f16, tag="wps")
                    nc.tensor.transpose(wtp[:, 0:tsz], w16[0:tsz, :],
                                        c_id[0:tsz, 0:tsz])
                    nc.vector.tensor_copy(wmT[:, t0:t0 + tsz],
                                          wtp[:, 0:tsz])

                # ---- w_flat [1, M*L] fp16 (m-major) via DMA, then DMA
                # partition-broadcast to all 128 partitions ----
                w_flat = wfp.tile([1, WCOLS], f16, tag="w_flat")
                nc.sync.dma_start(
                    _ap(w_flat[:], 0, [[L, M], [1, L]]), wmT[:])
                Wbc = wbcp.tile([128, WCOLS], f16, tag="Wbc")
                wf_bc = bass.AP(w_flat[:].tensor, w_flat[:].offset,
                                [[1, 1], [0, 128], [1, WCOLS]])
                nc.sync.dma_start(Wbc[:], wf_bc)

                state[b] = (kT, h_T, a_T, Wbc)

            def phase2a(b):
                kT, h_T, a_T, Wbc = state.pop(b)

                # t=0 cols of C: w[0, m] * Mv0[:, m]
                C = ccp.tile([128, WCOLS], f16, tag="C")
                t0w = _ap(Wbc[:], 0, [[L, M]])
                nc.vector.tensor_tensor(_ap(C[:], 0, [[L, M]]),
                                        c_Mv0[:], t0w, ALU.mult)

                for g in range(NGRP):
                    g0 = g * GCOLS
                    m0 = g * MGRP

                    # d0 = 1 - W*h  (reset cols: 0)
                    NW = unit.tile([128, SCOLS], f16, tag="NW")
                    nw_s = _ap(NW[:], 1, [[L + 1, MGRP], [1, L]])
                    wb_g = _ap(Wbc[:], g0, [[L, MGRP], [1, L]])
                    h_bc = _ap(h_T[:], 0, [[0, MGRP], [1, L]])
                    nc.vector.tensor_tensor(nw_s, wb_g, h_bc, ALU.mult)
                    nc.scalar.activation(nw_s, nw_s, AF.Copy,
                                         bias=1.0, scale=-1.0)
                    nc.vector.memset(_ap(NW[:], 0, [[L + 1, MGRP]]), 0.0)

                    # d1 = W*a  (reset cols: +Mv0 -> state resets to Mv0)
                    BN = unit.tile([128, SCOLS], f16, tag="BN")
                    bn_s = _ap(BN[:], 1, [[L + 1, MGRP], [1, L]])
                    a_bc = _ap(a_T[:], 0, [[0, MGRP], [1, L]])
                    nc.gpsimd.tensor_tensor(bn_s, wb_g, a_bc, ALU.mult)
                    nc.vector.tensor_copy(_ap(BN[:], 0, [[L + 1, MGRP]]),
                                          c_Mv0[:, m0:m0 + MGRP])

                    # scan: state = (d0 * state) + d1
                    Y = unit.tile([128, SCOLS], f16, tag="Y")
                    nc.vector.tensor_tensor_scan(Y[:], NW[:], BN[:], 0.0,
                                                 ALU.mult, ALU.add)

                    # G into C, m-major: C[m*L + t] = Y[m, t] * Wbc[m, t]
                    # (Y col j=t holds pre-update state for step t), t>=1
                    c_v = _ap(C[:], m0 * L + 1, [[L, MGRP], [1, L - 1]])
                    y_v = _ap(Y[:], 1, [[L + 1, MGRP], [1, L - 1]])
                    w_v = _ap(Wbc[:], g0 + 1, [[L, MGRP], [1, L - 1]])
                    nc.vector.tensor_tensor(c_v, y_v, w_v, ALU.mult)

                state2[b] = (kT, C)

            def phase2b(b):
                kT, C = state2.pop(b)
                fps = psF.tile([D, L], f32, tag="fps")
                for m in range(M):
                    nc.tensor.matmul(
                        fps[:], c_fWrT[:],
                        _ap(C[:], m * L, [[1, L]]),
                        start=(m == 0), stop=False,
                        skip_group_check=True)
                nc.tensor.matmul(fps[:], c_fWkT[:], kT,
                                 start=False, stop=True,
                                 skip_group_check=True)
                f_T = sm.tile([D, L], f16, tag="f_T")
                nc.scalar.activation(f_T[:], fps[:], AF.Tanh,
                                     bias=c_fb[:], scale=1.0)
                pps = psP.tile([1, L], f32, tag="pps")
                nc.tensor.matmul(pps[:], c_pWT[:], f_T[:])
                th_p = sm.tile([1, L], f16, tag="th_p")
                nc.scalar.activation(th_p[:], pps[:], AF.Tanh,
                                     bias=c_pb2[:], scale=0.5)
                nc.vector.tensor_scalar(p_row[0:1, b * L:(b + 1) * L],
                                        th_p[:], 0.5, 0.5,
                                        ALU.mult, ALU.add)

            for i in range(BLn + 2):
                if i == 0:
                    for j in range(min(3, BLn)):
                        gathers(j)
                if i < BLn:
                    phase1a(i)
                if i + 3 < BLn:
                    gathers(i + 3)
                if 1 <= i <= BLn:
                    phase2a(i - 1)
                if i >= 2:
                    phase2b(i - 2)

            nc.sync.dma_start(p_out[:, :],
                              _ap(p_row[:], 1, [[L, BLn], [1, L - 1]]))

    nc.compile()
    return nc


def _idx_table(ids):
    """ids [n, L] -> dma_gather idx table [128, n*NIDX/16] int16."""
    out = np.empty((128, ids.shape[0] * NIDX // 16), np.int16)
    for b in range(ids.shape[0]):
        pad = np.full(NIDX, -1, np.int16)
        pad[:L] = ids[b]
        tab = np.tile(pad.reshape(NIDX // 16, 16).T, (8, 1))
        out[:, b * (NIDX // 16):(b + 1) * (NIDX // 16)] = tab
    return out


def make_common(k_emb, v_emb, Mk, Mv0, e_W, e_b, a_W, a_b, f_W, f_b,
                p_W, p_b):
    return {
        "k_emb": np.asarray(k_emb, np.float16),
        "v_emb": np.asarray(v_emb, np.float16),
        "MkT": np.ascontiguousarray(np.asarray(Mk, np.float16).T),
        "eWT": np.ascontiguousarray(np.asarray(e_W, np.float16).T),
        "aWT": np.ascontiguousarray(np.asarray(a_W, np.float16).T),
        "fWrT": np.ascontiguousarray(np.asarray(f_W, np.float16)[:, :D].T),
        "fWkT": np.ascontiguousarray(np.asarray(f_W, np.float16)[:, D:].T),
        "pWT": np.ascontiguousarray(np.asarray(p_W, np.float16).T),
        "Mv0T16": np.ascontiguousarray(np.asarray(Mv0, np.float16).T),
        "ident": np.eye(D, dtype=np.float16),
        "eb2": (np.asarray(e_b, np.float32) / 2).reshape(D, 1),
        "a_b": np.asarray(a_b, np.float32).reshape(D, 1),
        "f_b": np.asarray(f_b, np.float32).reshape(D, 1),
        "pb2": (np.asarray(p_b, np.float32) / 2).reshape(1, 1),
    }


def kernel(skills, responses, k_emb, v_emb, Mk, Mv0,
           e_W, e_b, a_W, a_b, f_W, f_b, p_W, p_b):
    skills = np.asarray(skills)
    responses = np.asarray(responses)

    masked_r = responses * (responses > -1).astype(responses.dtype)
    x = (skills.astype(np.int64) + NS * masked_r.astype(np.int64))

    common = make_common(k_emb, v_emb, Mk, Mv0, e_W, e_b, a_W, a_b,
                         f_W, f_b, p_W, p_b)

    in_maps = []
    for c in range(NCORES):
        bsl = slice(c * BL, (c + 1) * BL)
        m = dict(common)
        m["kidx"] = _idx_table(skills[bsl])
        m["vidx"] = _idx_table(x[bsl])
        in_maps.append(m)

    nc = build_bass()
    global LAST_RESULTS
    res = run_bass_kernel_spmd(nc, in_maps, core_ids=list(range(NCORES)),
                               trace=TRACE)
    LAST_RESULTS = res
    out = np.concatenate([res.results[c]["p_out"] for c in range(NCORES)],
                         axis=0)
    return out.astype(np.float32)


# revision 20
# speedup vs baseline: 1.2787x; 1.0373x over previous
"""DKVMN forward Trainium2 Bass kernel (fp16 bulk path, v2).

Model (per sample): embeddings -> softmax attention w over M memory slots ->
sequential memory update Mv_t = Mv_{t-1} * (1 - w_t e_t^T) + w_t a_t^T ->
weighted read of PRE-update memory -> output MLP -> sigmoid.

Sharding: data-parallel over batch. B=64 across 8 cores -> 8 samples/core.
Tables + weights replicated. Each core returns [8, 199]; host concatenates.

v2 structure (per core; engine-balanced against the TimelineSim cost model):
- natural-form scan (no sign trick): state = (d0 * state) + d1 with
  d0 = 1 - W*e, d1 = W*a; m-blocks chained in one scan instr per group via
  reset columns (d0=0, d1=Mv0 -> state resets to Mv0).
- sigmoid via tanh: sigmoid(x) = 0.5*tanh(x/2)+0.5 so every Act func
  (Tanh/Exp/Copy) lives in one act table -> no LoadActFuncSet swaps.
  The output affine folds into cheap DVE tensor_scalar (4x mode) ops.
- engine split: scans+NW+G on DVE, BN on Pool, 1-We affine + Wbc
  PSUM->SBUF copies on Act, broadcast + all matmuls incl. the 50-term
  m-reduction on PE (Ldweights free, PSUM accumulation).
- software pipelining: emit P1(b+1) (gather/emb/softmax/Wbc) before
  P2(b) (scan chain + readout) so each engine's in-order stream always
  has sample b+1 front-end work before sample b's back-end waits.
"""
import sys

sys.path.insert(0, "/opt/trn_rl_repo")

import numpy as np

import concourse.bacc as bacc
import concourse.bass as bass
import concourse.tile as tile
from concourse import library_config, mybir
from concourse.bass_utils import run_bass_kernel_spmd

f32 = mybir.dt.float32
f16 = mybir.dt.float16
i16 = mybir.dt.int16
AF = mybir.ActivationFunctionType
ALU = mybir.AluOpType
AX = mybir.AxisListType

B, L, NS, D, M = 64, 200, 1000, 128, 50
NCORES = 8
BL = B // NCORES          # samples per core
NIDX = 256                # padded gather idxs per sample (L=200 real; must be %128)
MGRP = 10                 # m's per scan group
NGRP = M // MGRP          # 5 groups
GCOLS = MGRP * L          # 2000 w-cols per group
SCOLS = MGRP * (L + 1)    # 2010 scan cols (incl. reset col per m)
WCOLS = M * L             # 10000

TRACE = False
LAST_RESULTS = None


def _ap(t_ap, offset_add, free_dims):
    """Raw AP view: keep partition dim, replace free dims."""
    return bass.AP(t_ap.tensor, t_ap.offset + offset_add,
                   [t_ap.ap[0]] + free_dims)


def build_bass(n_samples=BL):
    BLn = n_samples
    nc = bacc.Bacc("TRN2", target_bir_lowering=False, debug=False,
                   num_devices=NCORES)

    def dram_in(name, shape, dtype=f32):
        return nc.dram_tensor(name, shape, dtype, kind="ExternalInput")

    k_emb = dram_in("k_emb", [NS, D], f16)
    v_emb = dram_in("v_emb", [2 * NS, D], f16)
    kidx = dram_in("kidx", [128, BLn * NIDX // 16], i16)
    vidx = dram_in("vidx", [128, BLn * NIDX // 16], i16)
    MkT = dram_in("MkT", [D, M], f16)
    eWT = dram_in("eWT", [D, D], f16)
    aWT = dram_in("aWT", [D, D], f16)
    fWrT = dram_in("fWrT", [D, D], f16)
    fWkT = dram_in("fWkT", [D, D], f16)
    pWT = dram_in("pWT", [D, 1], f16)
    Mv0T16 = dram_in("Mv0T16", [D, M], f16)
    ident = dram_in("ident", [D, D], f16)
    eb2 = dram_in("eb2", [D, 1])            # e_b / 2
    a_b = dram_in("a_b", [D, 1])
    f_b = dram_in("f_b", [D, 1])
    pb2 = dram_in("pb2", [1, 1])            # p_b / 2
    p_out = nc.dram_tensor("p_out", [BLn, L - 1], f32, kind="ExternalOutput")

    with tile.TileContext(nc) as tc:
        nc.gpsimd.load_library(library_config.mlp)
        with tc.tile_pool(name="const", bufs=1) as cpool, \
             tc.tile_pool(name="rows", bufs=6) as rpool, \
             tc.tile_pool(name="sm", bufs=2) as sm, \
             tc.tile_pool(name="wfp", bufs=2) as wfp, \
             tc.tile_pool(name="wbcp", bufs=2) as wbcp, \
             tc.tile_pool(name="unit", bufs=2) as unit, \
             tc.tile_pool(name="cc", bufs=2) as ccp, \
             tc.tile_pool(name="psE", bufs=1, space="PSUM") as psE, \
             tc.tile_pool(name="psA", bufs=1, space="PSUM") as psA, \
             tc.tile_pool(name="psF", bufs=2, space="PSUM") as psF, \
             tc.tile_pool(name="psP", bufs=1, space="PSUM") as psP, \
             tc.tile_pool(name="psW", bufs=1, space="PSUM") as psW:

            def cload(dram, shape, dtype=f32):
                t = cpool.tile(shape, dtype, tag=dram.name)
                nc.sync.dma_start(t[:], dram[(slice(None),) * len(shape)])
                return t

            c_kidx = cload(kidx, [128, BLn * NIDX // 16], i16)
            c_vidx = cload(vidx, [128, BLn * NIDX // 16], i16)
            c_MkT = cload(MkT, [D, M], f16)
            c_eWT = cload(eWT, [D, D], f16)
            c_aWT = cload(aWT, [D, D], f16)
            c_fWrT = cload(fWrT, [D, D], f16)
            c_fWkT = cload(fWkT, [D, D], f16)
            c_pWT = cload(pWT, [D, 1], f16)
            c_Mv0 = cload(Mv0T16, [D, M], f16)
            c_id = cload(ident, [D, D], f16)
            c_eb2 = cload(eb2, [D, 1])
            c_ab = cload(a_b, [D, 1])
            c_fb = cload(f_b, [D, 1])
            c_pb2 = cload(pb2, [1, 1])

            p_row = sm.tile([1, BLn * L], f32, tag="p_row")

            state = {}
            state2 = {}

            gat = {}

            def gathers(b):
                isl = slice(b * (NIDX // 16), (b + 1) * (NIDX // 16))
                # ---- transposed gathers: directly [D, t] fp16 ----
                kT3 = rpool.tile([128, 1, NIDX], f16, tag="kT3")
                nc.gpsimd.dma_gather(kT3[:], k_emb[:, :], c_kidx[:, isl],
                                     num_idxs=NIDX, num_idxs_reg=L,
                                     elem_size=D, transpose=True)
                vT3 = rpool.tile([128, 1, NIDX], f16, tag="vT3")
                nc.gpsimd.dma_gather(vT3[:], v_emb[:, :], c_vidx[:, isl],
                                     num_idxs=NIDX, num_idxs_reg=L,
                                     elem_size=D, transpose=True)
                gat[b] = (kT3, vT3)

            def phase1a(b):
                kT3, vT3 = gat.pop(b)
                kT = _ap(kT3[:], 0, [[1, L]])
                vT = _ap(vT3[:], 0, [[1, L]])

                # ---- h = sigmoid(e_W v + e_b) via tanh; a = tanh(...) ----
                eps = psE.tile([D, L], f32, tag="eps")
                nc.tensor.matmul(eps[:], c_eWT[:], vT)
                th_e = sm.tile([D, L], f16, tag="th_e")
                nc.scalar.activation(th_e[:], eps[:], AF.Tanh,
                                     bias=c_eb2[:], scale=0.5)
                h_T = sm.tile([D, L], f16, tag="h_T")
                nc.vector.tensor_scalar(h_T[:], th_e[:], 0.5, 0.5,
                                        ALU.mult, ALU.add)
                aps = psA.tile([D, L], f32, tag="aps")
                nc.tensor.matmul(aps[:], c_aWT[:], vT)
                a_T = sm.tile([D, L], f16, tag="a_T")
                nc.scalar.activation(a_T[:], aps[:], AF.Tanh,
                                     bias=c_ab[:], scale=1.0)

                # ---- w softmax (f32 psum) -> fp16 [m, t] ----
                wmT = sm.tile([M, L], f16, tag="wmT")
                for tb in range(2):
                    t0 = tb * 128
                    tsz = min(128, L - t0)
                    wps = psW.tile([128, M], f32, tag="wps")
                    nc.tensor.matmul(wps[0:tsz, :],
                                     _ap(kT3[:], t0, [[1, tsz]]),
                                     c_MkT[:])
                    negmax = sm.tile([128, 1], f32, tag="negmax")
                    nc.vector.tensor_reduce(negmax[0:tsz, :], wps[0:tsz, :],
                                            AX.X, ALU.max, negate=True)
                    wexp = sm.tile([128, M], f32, tag="wexp")
                    nc.scalar.activation(wexp[0:tsz, :], wps[0:tsz, :],
                                         AF.Exp, bias=negmax[0:tsz, :],
                                         scale=1.0)
                    ssum = sm.tile([128, 1], f32, tag="ssum")
                    nc.vector.tensor_reduce(ssum[0:tsz, :], wexp[0:tsz, :],
                                            AX.X, ALU.add)
                    rcp = sm.tile([128, 1], f32, tag="rcp")
                    nc.vector.reciprocal(rcp[0:tsz, :], ssum[0:tsz, :])
                    w16 = sm.tile([128, M], f16, tag="w16")
                    nc.vector.tensor_scalar_mul(w16[0:tsz, :], wexp[0:tsz, :],
                                                rcp[0:tsz, :])
                    wtp = psW.tile([M, 128], f16, tag="wps")
                    nc.tensor.transpose(wtp[:, 0:tsz], w16[0:tsz, :],
                                        c_id[0:tsz, 0:tsz])
                    nc.vector.tensor_copy(wmT[:, t0:t0 + tsz],
                                          wtp[:, 0:tsz])

                # ---- w_flat [1, M*L] fp16 (m-major) via DMA, then DMA
                # partition-broadcast to all 128 partitions ----
                w_flat = wfp.tile([1, WCOLS], f16, tag="w_flat")
                nc.sync.dma_start(
                    _ap(w_flat[:], 0, [[L, M], [1, L]]), wmT[:])
                Wbc = wbcp.tile([128, WCOLS], f16, tag="Wbc")
                wf_bc = bass.AP(w_flat[:].tensor, w_flat[:].offset,
                                [[1, 1], [0, 128], [1, WCOLS]])
                nc.sync.dma_start(Wbc[:], wf_bc)

                state[b] = (kT, h_T, a_T, Wbc)

            def phase2a(b):
                kT, h_T, a_T, Wbc = state.pop(b)

                # t=0 cols of C: w[0, m] * Mv0[:, m]
                C = ccp.tile([128, WCOLS], f16, tag="C")
                t0w = _ap(Wbc[:], 0, [[L, M]])
                nc.vector.tensor_tensor(_ap(C[:], 0, [[L, M]]),
                                        c_Mv0[:], t0w, ALU.mult)

                for g in range(NGRP):
                    g0 = g * GCOLS
                    m0 = g * MGRP

                    # d0 = 1 - W*h  (reset cols: 0)
                    NW = unit.tile([128, SCOLS], f16, tag="NW")
                    nw_s = _ap(NW[:], 1, [[L + 1, MGRP], [1, L]])
                    wb_g = _ap(Wbc[:], g0, [[L, MGRP], [1, L]])
                    h_bc = _ap(h_T[:], 0, [[0, MGRP], [1, L]])
                    nc.vector.tensor_tensor(nw_s, wb_g, h_bc, ALU.mult)
                    nc.scalar.activation(nw_s, nw_s, AF.Copy,
                                         bias=1.0, scale=-1.0)
                    nc.vector.memset(_ap(NW[:], 0, [[L + 1, MGRP]]), 0.0)

                    # d1 = W*a  (reset cols: +Mv0 -> state resets to Mv0)
                    BN = unit.tile([128, SCOLS], f16, tag="BN")
                    bn_s = _ap(BN[:], 1, [[L + 1, MGRP], [1, L]])
                    a_bc = _ap(a_T[:], 0, [[0, MGRP], [1, L]])
                    nc.gpsimd.tensor_tensor(bn_s, wb_g, a_bc, ALU.mult)
                    nc.vector.tensor_copy(_ap(BN[:], 0, [[L + 1, MGRP]]),
                                          c_Mv0[:, m0:m0 + MGRP])

                    # scan: state = (d0 * state) + d1
                    Y = unit.tile([128, SCOLS], f16, tag="Y")
                    nc.vector.tensor_tensor_scan(Y[:], NW[:], BN[:], 0.0,
                                                 ALU.mult, ALU.add)

                    # G into C, m-major: C[m*L + t] = Y[m, t] * Wbc[m, t]
                    # (Y col j=t holds pre-update state for step t), t>=1
                    c_v = _ap(C[:], m0 * L + 1, [[L, MGRP], [1, L - 1]])
                    y_v = _ap(Y[:], 1, [[L + 1, MGRP], [1, L - 1]])
                    w_v = _ap(Wbc[:], g0 + 1, [[L, MGRP], [1, L - 1]])
                    nc.vector.tensor_tensor(c_v, y_v, w_v, ALU.mult)

                state2[b] = (kT, C)

            def phase2b(b):
                kT, C = state2.pop(b)
                fps = psF.tile([D, L], f32, tag="fps")
                for m in range(M):
                    nc.tensor.matmul(
                        fps[:], c_fWrT[:],
                        _ap(C[:], m * L, [[1, L]]),
                        start=(m == 0), stop=False,
                        skip_group_check=True)
                nc.tensor.matmul(fps[:], c_fWkT[:], kT,
                                 start=False, stop=True,
                                 skip_group_check=True)
                f_T = sm.tile([D, L], f16, tag="f_T")
                nc.scalar.activation(f_T[:], fps[:], AF.Tanh,
                                     bias=c_fb[:], scale=1.0)
                pps = psP.tile([1, L], f32, tag="pps")
                nc.tensor.matmul(pps[:], c_pWT[:], f_T[:])
                th_p = sm.tile([1, L], f16, tag="th_p")
                nc.scalar.activation(th_p[:], pps[:], AF.Tanh,
                                     bias=c_pb2[:], scale=0.5)
                nc.vector.tensor_scalar(p_row[0:1, b * L:(b + 1) * L],
                                        th_p[:], 0.5, 0.5,
                                        ALU.mult, ALU.add)

            for i in range(BLn + 2):
                if i == 0:
                    for j in range(min(3, BLn)):
                        gathers(j)
                if i < BLn:
                    phase1a(i)
                if 1 <= i <= BLn:
                    phase2a(i - 1)
                if i + 3 < BLn:
                    gathers(i + 3)
                if i >= 2:
                    phase2b(i - 2)

            nc.sync.dma_start(p_out[:, :],
                              _ap(p_row[:], 1, [[L, BLn], [1, L - 1]]))

    nc.compile()
    return nc


def _idx_table(ids):
    """ids [n, L] -> dma_gather idx table [128, n*NIDX/16] int16."""
    out = np.empty((128, ids.shape[0] * NIDX // 16), np.int16)
    for b in range(ids.shape[0]):
        pad = np.full(NIDX, -1, np.int16)
        pad[:L] = ids[b]
        tab = np.tile(pad.reshape(NIDX // 16, 16).T, (8, 1))
        out[:, b * (NIDX // 16):(b + 1) * (NIDX // 16)] = tab
    return out


def make_common(k_emb, v_emb, Mk, Mv0, e_W, e_b, a_W, a_b, f_W, f_b,
                p_W, p_b):
    return {
        "k_emb": np.asarray(k_emb, np.float16),
        "v_emb": np.asarray(v_emb, np.float16),
        "MkT": np.ascontiguousarray(np.asarray(Mk, np.float16).T),
        "eWT": np.ascontiguousarray(np.asarray(e_W, np.float16).T),
        "aWT": np.ascontiguousarray(np.asarray(a_W, np.float16).T),
        "fWrT": np.ascontiguousarray(np.asarray(f_W, np.float16)[:, :D].T),
        "fWkT": np.ascontiguousarray(np.asarray(f_W, np.float16)[:, D:].T),
        "pWT": np.ascontiguousarray(np.asarray(p_W, np.float16).T),
        "Mv0T16": np.ascontiguousarray(np.asarray(Mv0, np.float16).T),
        "ident": np.eye(D, dtype=np.float16),
        "eb2": (np.asarray(e_b, np.float32) / 2).reshape(D, 1),
        "a_b": np.asarray(a_b, np.float32).reshape(D, 1),
        "f_b": np.asarray(f_b, np.float32).reshape(D, 1),
        "pb2": (np.asarray(p_b, np.float32) / 2).reshape(1, 1),
    }


def kernel(skills, responses, k_emb, v_emb, Mk, Mv0,
           e_W, e_b, a_W, a_b, f_W, f_b, p_W, p_b):
    skills = np.asarray(skills)
    responses = np.asarray(responses)

    masked_r = responses * (responses > -1).astype(responses.dtype)
    x = (skills.astype(np.int64) + NS * masked_r.astype(np.int64))

    common = make_common(k_emb, v_emb, Mk, Mv0, e_W, e_b, a_W, a_b,
                         f_W, f_b, p_W, p_b)

    in_maps = []
    for c in range(NCORES):
        bsl = slice(c * BL, (c + 1) * BL)
        m = dict(common)
        m["kidx"] = _idx_table(skills[bsl])
        m["vidx"] = _idx_table(x[bsl])
        in_maps.append(m)

    nc = build_bass()
    global LAST_RESULTS
    res = run_bass_kernel_spmd(nc, in_maps, core_ids=list(range(NCORES)),
                               trace=TRACE)
    LAST_RESULTS = res
    out = np.concatenate([res.results[c]["p_out"] for c in range(NCORES)],
                         axis=0)
    return out.astype(np.float32)
